# revision 7
# baseline (speedup 1.0000x reference)
"""Trainium2 Bass kernel for additive (Bahdanau) attention GNN message passing.

score[n, m] = v . tanh(a[n] + b[m]),  a = x1 @ W1.T, b = x2 @ W2.T + bc
w = softmax(score, axis=n);  ctx[m] = w[:, m].T @ x1
out = tanh(concat([att, ctx_s, ctx_e]) @ W_lin.T + b_lin)

Sharding: attender dim M=1024 split across 8 cores (128 each); attendees and
params replicated. No collectives.

The per-(n,m,h) tanh (25M ACT elems/core naively) is replaced by a separable
harmonic expansion  tanh(s) ~ sum_k alpha_k sin(k w0 s)  with
sin(k w0 (a+b)) = sin(k w0 a) cos(k w0 b) + cos(k w0 a) sin(k w0 b),
turning the O(N*M*H) work into PE matmuls contracting h per harmonic.

HW ACT Sin is only valid for |arg| < ~pi, so only sin(.5 w0 x) and sin(w0 x)
run on ACT; everything else comes from trig identities in bf16:
  c1 = 1-2*Sq(u1), c2 = 1-2*Sq(s1), s2 = 2c1*s1, c3 = c1*(2c2-1),
  s3 = 2c1*s2-s1, s4 = 2c2*s2, c4 = 1-2*Sq(s2), s6 = 2c3*s3, c6 = 2*Sq(c3)-1
(Squares on ACT for the a-side, products on DVE; the small b-side chain runs
on gpsimd.)  v*alpha_k folds into the b-side tiles via per-partition
tensor_scalar; v, bc, alpha, b_lin enter as single-row DMAs expanded by
rank-1 PE matmuls (column DMAs are descriptor-bound and slow).

Score matmuls run "m-stationary": lhsT = weighted b tile [h, m], rhs =
a-basis tile [h, 512-wide n block] -> PSUM score_mT [m, n], 6 wide matmuls
per harmonic instead of 24 narrow ones.  PSUM accumulation: start=True only
on each bank's first matmul (start clears the whole bank's has_written
bits).  E = exp(score_mT) is PE-transposed back to [n, m] chunks for the
ctx matmuls; softmax sums fall out of a ones column in the attendee image.
The final linear runs fully in bf16.
"""

import numpy as np
from ml_dtypes import bfloat16

import concourse.bass as bass
import concourse.tile as tile
from concourse import bacc, masks, mybir
from concourse.bass_utils import run_bass_kernel_spmd

F32 = mybir.dt.float32
BF16 = mybir.dt.bfloat16
AF = mybir.ActivationFunctionType
OP = mybir.AluOpType

H = 128      # hidden
A = 256      # attention (output) size
N_S = 1024   # attendee statements
N_E = 512    # attendee EREs
M = 1024     # attenders
NC = 8       # cores
ML = M // NC # attenders per core
NT = N_S + N_E  # 1536
NCH = NT // 128  # 12 chunks of attendees
CW = 129     # x-image chunk width: 128 attendee cols + a ones column
X16W = NCH * CW  # 1548
IMG2W = X16W + 3 * A  # x16 | wlinT16

W0 = 0.267059
KS = (1, 2, 3, 4, 6)
ALPHA = (1.17663, 0.12087, 0.17747, 0.13768, 0.13409)

# a-side tiles are emitted in two column parts so downstream consumers can
# start on part 0 while part 1 is still being produced
PARTS = ((0, 512), (512, 1536))
BLOCKS = ((0, 512, 0), (512, 1024, 0), (1024, 1536, 1))  # (lo, hi, set)

_CACHE = {}


def _build():
    nc = bacc.Bacc(
        "TRN2", target_bir_lowering=False, debug=False, num_devices=NC
    )

    d_imgA = nc.dram_tensor("imgA", [128, 640], BF16, kind="ExternalInput").ap()
    d_imgB = nc.dram_tensor("imgB", [128, NT], BF16, kind="ExternalInput").ap()
    d_crow = nc.dram_tensor("crow", [1, 1536], BF16, kind="ExternalInput").ap()
    d_img2 = nc.dram_tensor("img2", [128, IMG2W], BF16, kind="ExternalInput").ap()
    d_out = nc.dram_tensor("out", [ML, A], F32, kind="ExternalOutput").ap()

    with tile.TileContext(nc) as tc:
        _emit(nc, tc, d_imgA, d_imgB, d_crow, d_img2, d_out)

    nc.compile()
    return nc


def _emit(nc, tc, d_imgA, d_imgB, d_crow, d_img2, d_out):
    from contextlib import ExitStack

    ctx = ExitStack()
    with ctx:
        const = ctx.enter_context(tc.tile_pool(name="const", bufs=1))
        bpool = ctx.enter_context(tc.tile_pool(name="bpool", bufs=1))
        apool = ctx.enter_context(tc.tile_pool(name="apool", bufs=1))
        ps_a = ctx.enter_context(
            tc.tile_pool(name="ps_a", bufs=1, space=bass.MemorySpace.PSUM))
        ps_score = ctx.enter_context(
            tc.tile_pool(name="ps_score", bufs=1, space=bass.MemorySpace.PSUM))
        ps_small = ctx.enter_context(
            tc.tile_pool(name="ps_small", bufs=1, space=bass.MemorySpace.PSUM))

        # ---- init + ACT trig-table warm ----
        ident16 = const.tile([128, 128], BF16)
        masks.make_identity(nc, ident16[:])
        ones16 = const.tile([1, 128], BF16)
        nc.gpsimd.memset(ones16[:], 1.0)
        scratch = const.tile([128, 1], F32)
        nc.gpsimd.memset(scratch[:], 0.0)
        nc.scalar.activation(scratch[:], scratch[:], AF.Sin)

        # ---- DMAs (packed images; few triggers, wide rows) ----
        sb_imgA = const.tile([128, 640], BF16)     # wT16 | attT16
        nc.sync.dma_start(sb_imgA[:], d_imgA[:, :])
        sb_imgB = const.tile([128, NT], BF16)      # stmtsT | eresT
        nc.sync.dma_start(sb_imgB[:], d_imgB[:, :])
        sb_crow = const.tile([1, 1536], BF16)      # vs|ve|bcs|bce|alpha|blin
        nc.sync.dma_start(sb_crow[0:1, :], d_crow[0:1, :])
        sb_img2 = const.tile([128, IMG2W], BF16)   # x16 | wlinT16
        nc.gpsimd.dma_start(sb_img2[:], d_img2[:, :])

        wT = sb_imgA[:, 0:512]
        attT16 = sb_imgA[:, 512:640]
        crow = lambda r, n: sb_crow[0:1, r * 256:r * 256 + n]
        x16 = sb_img2[:, 0:X16W]
        wlin = lambda j: sb_img2[:, X16W + j * A:X16W + (j + 1) * A]

        # ---- front PE: bT both sets + bias/valpha rank-1, all one bank ----
        ps_bT = ps_small.tile([128, 272], F32, tag="ctx", name="ps_bT")
        nc.tensor.matmul(ps_bT[:, 0:ML], wT[:, 128:256], attT16,
                         start=True, stop=False, skip_group_check=True)
        nc.tensor.matmul(ps_bT[:, ML:2 * ML], wT[:, 384:512], attT16,
                         start=False, stop=False, skip_group_check=True)
        nc.tensor.matmul(ps_bT[:, 0:ML], crow(2, 128), ones16[0:1, :],
                         start=False, stop=False, skip_group_check=True)
        nc.tensor.matmul(ps_bT[:, ML:2 * ML], crow(3, 128), ones16[0:1, :],
                         start=False, stop=False, skip_group_check=True)
        nc.tensor.matmul(ps_bT[:, 256:261], crow(0, 128), crow(4, len(KS)),
                         start=False, stop=False, skip_group_check=True)
        nc.tensor.matmul(ps_bT[:, 261:266], crow(1, 128), crow(4, len(KS)),
                         start=False, stop=True, skip_group_check=True)
        sb_valpha = const.tile([128, 2 * len(KS)], F32)
        nc.vector.tensor_copy(sb_valpha[:], ps_bT[:, 256:266])

        # ---- aT for both sets -> [128, 1536] PSUM (bank-aligned pieces) ----
        ps_aT = ps_a.tile([128, NT], F32, tag="aT", name="ps_aT")
        nc.tensor.matmul(ps_aT[:, 0:512], wT[:, 0:128], sb_imgB[:, 0:512],
                         start=True, stop=True)
        nc.tensor.matmul(ps_aT[:, 512:1024], wT[:, 0:128],
                         sb_imgB[:, 512:1024], start=True, stop=True)
        nc.tensor.matmul(ps_aT[:, 1024:1536], wT[:, 256:384],
                         sb_imgB[:, 1024:1536], start=True, stop=True)

        # att + b_lin parts of the final linear (anytime before epilogue)
        ps_out = ps_small.tile([128, 400], F32, tag="out")
        nc.tensor.matmul(ps_out[:, 0:A], attT16, wlin(0),
                         start=True, stop=False, skip_group_check=True)
        nc.tensor.matmul(ps_out[:, 0:A], ones16[0:1, :], crow(5, A),
                         start=False, stop=False, skip_group_check=True)

        # ---- b-side basis: ACT small sins, then gpsimd identity chain ----
        # u1b reads cols 0:266 (incl. junk valpha cols) to order ACT's first
        # PSUM-bank read after the PE writes (collision avoidance).
        u1b = bpool.tile([128, 272], BF16, name="u1b")
        nc.scalar.activation(u1b[:, 0:266], ps_bT[:, 0:266], AF.Sin,
                             scale=0.5 * W0)
        bt = {k: bpool.tile([128, 512], BF16, name=f"bt{k}") for k in KS}
        nc.scalar.activation(bt[1][:, 0:256], ps_bT[:, 0:256], AF.Sin,
                             scale=W0)
        s1b = bt[1][:, 0:256]

        def btmp(nm):
            return bpool.tile([128, 256], BF16, name=nm)[:]

        g = nc.gpsimd
        q1b = btmp("q1b")
        g.tensor_tensor(q1b, u1b[:, 0:256], u1b[:, 0:256], OP.mult)
        c1b = bt[1][:, 256:512]
        nc.vector.tensor_scalar(c1b, q1b, -2.0, 1.0, OP.mult, OP.add)
        c1twob = btmp("c1twob")
        nc.vector.tensor_scalar_mul(c1twob, c1b, 2.0)
        g.tensor_tensor(bt[2][:, 0:256], c1twob, s1b, OP.mult)     # s2b
        qs1b = btmp("qs1b")
        g.tensor_tensor(qs1b, s1b, s1b, OP.mult)
        c2b = bt[2][:, 256:512]
        nc.vector.tensor_scalar(c2b, qs1b, -2.0, 1.0, OP.mult, OP.add)
        c2twob = btmp("c2twob")
        nc.vector.tensor_scalar_mul(c2twob, c2b, 2.0)
        c2mb = btmp("c2mb")
        nc.vector.tensor_scalar_sub(c2mb, c2twob, 1.0)
        g.tensor_tensor(bt[3][:, 256:512], c1b, c2mb, OP.mult)     # c3b
        s3tb = btmp("s3tb")
        g.tensor_tensor(s3tb, c1twob, bt[2][:, 0:256], OP.mult)
        g.tensor_tensor(bt[3][:, 0:256], s3tb, s1b, OP.subtract)   # s3b
        g.tensor_tensor(bt[4][:, 0:256], c2twob, bt[2][:, 0:256],
                        OP.mult)                                   # s4b
        qs2b = btmp("qs2b")
        g.tensor_tensor(qs2b, bt[2][:, 0:256], bt[2][:, 0:256], OP.mult)
        nc.vector.tensor_scalar(bt[4][:, 256:512], qs2b, -2.0, 1.0,
                                OP.mult, OP.add)                   # c4b
        c3twob = btmp("c3twob")
        nc.vector.tensor_scalar_mul(c3twob, bt[3][:, 256:512], 2.0)
        g.tensor_tensor(bt[6][:, 0:256], c3twob, bt[3][:, 0:256],
                        OP.mult)                                   # s6b
        qc3b = btmp("qc3b")
        g.tensor_tensor(qc3b, bt[3][:, 256:512], bt[3][:, 256:512], OP.mult)
        nc.vector.tensor_scalar(bt[6][:, 256:512], qc3b, 2.0, -1.0,
                                OP.mult, OP.add)                   # c6b

        # ---- b-side weighting on DVE: w = valpha[set] * tile, both trig
        # blocks of a set in one strided instr ----
        wsc = {k: bpool.tile([128, 512], BF16, name=f"wsc{k}") for k in KS}
        for ki, k in enumerate(KS):
            src3 = bt[k][:].rearrange("p (t s m) -> p t s m", t=2, s=2)
            dst3 = wsc[k][:].rearrange("p (t s m) -> p t s m", t=2, s=2)
            for st in range(2):
                nc.vector.tensor_scalar(
                    dst3[:, :, st, :], src3[:, :, st, :],
                    sb_valpha[:, st * len(KS) + ki:st * len(KS) + ki + 1],
                    None, OP.mult)

        # ---- a-side basis: ACT sins+squares, DVE identity chain ----
        at = {}
        for nm in ("u1a", "s1", "qu1", "qs1", "c1", "c1two", "c2", "c2two",
                   "c2m", "s2", "c3", "s3t", "s3", "s4", "qs2", "c4",
                   "c3two", "s6", "qc3", "c6"):
            at[nm] = apool.tile([128, NT], BF16, name=nm)

        def act2(out, in_, func, scale=1.0):
            for lo, hi in PARTS:
                nc.scalar.activation(out[:, lo:hi], in_[:, lo:hi], func,
                                     scale=scale)

        def dve_ts2(out, in_, s1_, s2_, op0, op1=None):
            for lo, hi in PARTS:
                if op1 is None:
                    nc.vector.tensor_scalar(out[:, lo:hi], in_[:, lo:hi],
                                            s1_, None, op0)
                else:
                    nc.vector.tensor_scalar(out[:, lo:hi], in_[:, lo:hi],
                                            s1_, s2_, op0, op1)

        def dve_tt2(out, in0, in1, op):
            for lo, hi in PARTS:
                nc.vector.tensor_tensor(out[:, lo:hi], in0[:, lo:hi],
                                        in1[:, lo:hi], op)

        act2(at["u1a"][:], ps_aT[:], AF.Sin, 0.5 * W0)
        act2(at["s1"][:], ps_aT[:], AF.Sin, W0)
        act2(at["qu1"][:], at["u1a"][:], AF.Square)
        act2(at["qs1"][:], at["s1"][:], AF.Square)
        dve_ts2(at["c1"][:], at["qu1"][:], -2.0, 1.0, OP.mult, OP.add)
        dve_ts2(at["c1two"][:], at["c1"][:], 2.0, None, OP.mult)
        dve_ts2(at["c2"][:], at["qs1"][:], -2.0, 1.0, OP.mult, OP.add)
        dve_tt2(at["s2"][:], at["c1two"][:], at["s1"][:], OP.mult)
        dve_ts2(at["c2two"][:], at["c2"][:], 2.0, None, OP.mult)
        dve_ts2(at["c2m"][:], at["c2two"][:], 1.0, None, OP.subtract)
        dve_tt2(at["c3"][:], at["c1"][:], at["c2m"][:], OP.mult)
        dve_tt2(at["s3t"][:], at["c1two"][:], at["s2"][:], OP.mult)
        dve_tt2(at["s3"][:], at["s3t"][:], at["s1"][:], OP.subtract)
        dve_tt2(at["s4"][:], at["c2two"][:], at["s2"][:], OP.mult)
        act2(at["qs2"][:], at["s2"][:], AF.Square)
        dve_ts2(at["c4"][:], at["qs2"][:], -2.0, 1.0, OP.mult, OP.add)
        dve_ts2(at["c3two"][:], at["c3"][:], 2.0, None, OP.mult)
        dve_tt2(at["s6"][:], at["c3two"][:], at["s3"][:], OP.mult)
        act2(at["qc3"][:], at["c3"][:], AF.Square)
        dve_ts2(at["c6"][:], at["qc3"][:], 2.0, -1.0, OP.mult, OP.add)

        AT = {1: ("s1", "c1"), 2: ("s2", "c2"), 3: ("s3", "c3"),
              4: ("s4", "c4"), 6: ("s6", "c6")}

        # ---- scores, m-stationary: ps_sc[m, n] ----
        ps_sc = ps_score.tile([128, NT], F32)
        for ki, k in enumerate(KS):
            s_nm, c_nm = AT[k]
            for trig in (0, 1):
                # sin_a pairs with cos_b weights and vice versa
                rhs_t = at[s_nm] if trig == 0 else at[c_nm]
                for lo, hi, st in BLOCKS:
                    lh = wsc[k][:, 256 * (1 - trig) + st * 128:
                                256 * (1 - trig) + st * 128 + 128]
                    nc.tensor.matmul(
                        ps_sc[:, lo:hi], lh, rhs_t[:, lo:hi],
                        start=(ki == 0 and trig == 0),
                        stop=(ki == len(KS) - 1 and trig == 1),
                        skip_group_check=True)

        # ---- epilogue ----
        E_mT = apool.tile([128, NT], BF16, name="E_mT")
        for lo, hi, _ in BLOCKS:
            nc.scalar.activation(E_mT[:, lo:hi], ps_sc[:, lo:hi], AF.Exp)

        # transpose E back to [n, m] chunks (PE), copy PSUM->SBUF per bank
        ps_tr2 = ps_a.tile([128, 2 * NT], BF16, tag="aT", name="ps_tr2")
        sb_E = apool.tile([128, NT], BF16, name="sb_E")
        for c in range(NCH):
            nc.tensor.matmul(ps_tr2[:, c * 128:(c + 1) * 128],
                             E_mT[:, c * 128:(c + 1) * 128], ident16[:],
                             is_transpose=True)
            if c == 7:
                nc.vector.tensor_copy(sb_E[:, 0:1024], ps_tr2[:, 0:1024])
        nc.vector.tensor_copy(sb_E[:, 1024:1536], ps_tr2[:, 1024:1536])

        # ctx (+softmax sums via ones column): stmt set -> ps_ctx bank,
        # ere set -> a region of the ps_out bank (avoids read/write
        # collisions between the two regions' consumers)
        ps_ctx = ps_small.tile([128, 272], F32, tag="ctx", name="ps_ctx")
        for c in range(8):
            nc.tensor.matmul(ps_ctx[:, 0:CW],
                             sb_E[:, c * 128:(c + 1) * 128],
                             x16[:, c * CW:(c + 1) * CW],
                             start=(c == 0), stop=(c == 7))
        for c in range(8, 12):
            nc.tensor.matmul(ps_out[:, 264:264 + CW],
                             sb_E[:, c * 128:(c + 1) * 128],
                             x16[:, c * CW:(c + 1) * CW],
                             start=False, stop=False, skip_group_check=True)

        sb_recip = apool.tile([128, 2], F32, name="recip")
        nc.vector.reciprocal(sb_recip[:, 0:1], ps_ctx[:, H:H + 1])
        nc.vector.reciprocal(sb_recip[:, 1:2], ps_out[:, 392:393])
        sb_ctx = apool.tile([128, 2 * H], BF16, name="sb_ctx")
        nc.vector.tensor_scalar(sb_ctx[:, 0:H], ps_ctx[:, 0:H],
                                sb_recip[:, 0:1], None, OP.mult)
        nc.vector.tensor_scalar(sb_ctx[:, H:2 * H], ps_out[:, 264:392],
                                sb_recip[:, 1:2], None, OP.mult)

        ps_tr3 = ps_a.tile([128, 2 * NT], BF16, tag="aT", name="ps_tr3")
        nc.tensor.matmul(ps_tr3[:, 0:128], sb_ctx[:, 0:H], ident16[:],
                         is_transpose=True)
        nc.tensor.matmul(ps_tr3[:, 128:256], sb_ctx[:, H:2 * H], ident16[:],
                         is_transpose=True)
        sb_ctxT = apool.tile([128, 2 * H], BF16, name="sb_ctxT")
        nc.vector.tensor_copy(sb_ctxT[:], ps_tr3[:, 0:256])

        nc.tensor.matmul(ps_out[:, 0:A], sb_ctxT[:, 0:H], wlin(1),
                         start=False, stop=False, skip_group_check=True)
        nc.tensor.matmul(ps_out[:, 0:A], sb_ctxT[:, H:2 * H], wlin(2),
                         start=False, stop=True, skip_group_check=True)

        sb_out = apool.tile([128, A], F32, name="sb_out")
        nc.scalar.activation(sb_out[:], ps_out[:, 0:A], AF.Tanh)
        nc.sync.dma_start(d_out[:, :], sb_out[:])


def _get_nc():
    if "nc" not in _CACHE:
        _CACHE["nc"] = _build()
    return _CACHE["nc"]


def _prep_inputs(inputs):
    """Host-side layout prep: transposes / bf16 casts / image packing."""
    f = {k: np.ascontiguousarray(np.asarray(v, np.float32))
         for k, v in inputs.items()}
    stmts, eres = f["attendee_stmts"], f["attendee_eres"]
    ws, we, wlin = f["Ws_concat"], f["We_concat"], f["W_lin"]

    imgB = np.concatenate([stmts.T, eres.T], axis=1).astype(bfloat16)

    crow = np.zeros((1, 1536), np.float32)
    crow[0, 0:128] = f["vs_single"]
    crow[0, 256:384] = f["ve_single"]
    crow[0, 512:640] = f["bs_concat"]
    crow[0, 768:896] = f["be_concat"]
    crow[0, 1024:1024 + len(KS)] = np.asarray(ALPHA, np.float32)
    crow[0, 1280:1536] = f["b_lin"]

    x = np.empty((128, X16W), np.float32)
    for c in range(8):
        x[:, c * CW:c * CW + H] = stmts[c * 128:(c + 1) * 128]
        x[:, c * CW + H] = 1.0
    for c in range(8, 12):
        x[:, c * CW:c * CW + H] = eres[(c - 8) * 128:(c - 7) * 128]
        x[:, c * CW + H] = 1.0
    wlinT = np.concatenate(
        [wlin[:, 0:H].T, wlin[:, H:2 * H].T, wlin[:, 2 * H:3 * H].T], axis=1)
    img2 = np.concatenate([x, wlinT], axis=1).astype(bfloat16)

    wT = np.concatenate(
        [ws[:, :H].T, ws[:, H:].T, we[:, :H].T, we[:, H:].T], axis=1)

    shared = {
        "imgB": np.ascontiguousarray(imgB),
        "crow": np.ascontiguousarray(crow.astype(bfloat16)),
        "img2": np.ascontiguousarray(img2),
    }
    att = f["attender"]
    in_maps = []
    for i in range(NC):
        imgA = np.concatenate(
            [wT, att[i * ML:(i + 1) * ML].T], axis=1).astype(bfloat16)
        in_maps.append(dict(shared, imgA=np.ascontiguousarray(imgA)))
    return in_maps


def kernel(**inputs) -> np.ndarray:
    nc = _get_nc()
    in_maps = _prep_inputs(inputs)
    res = run_bass_kernel_spmd(nc, in_maps, list(range(NC)))
    return np.concatenate([res.results[i]["out"] for i in range(NC)], axis=0)
